# revision 5
# baseline (speedup 1.0000x reference)
"""ColorHistogramLoss Trainium2 kernel (v2: DVE masks + PE matmul-sums + ACT).

Problem: loss = mean(|hist(input) - hist(target)|) with 64-bin histograms
per (batch, channel) over [-1, 1), inputs [32, 3, 512, 512] f32.

Strategy (8 cores, data-parallel over batch, 4 batches/core):
  - Binning: w = bf16_rne(v*(63/128) + (191/128 - 2^-8)). The -2^-8 pre-bias
    turns bf16 round-to-nearest into floor onto the 2^-7 grid of [1,2), so
    (w >= 1 + j/64) reproduces searchsorted binning exactly (boundary-rounding
    differences ~1e-5 of elements, loss rel-err ~1e-4).
  - CDF counts per edge j, split across engines (measured rates):
      * PE_EDGES (45): DVE writes the 0/1 mask at 4x mode (~2.2us/group);
        the tensor engine sums masks via 96 accumulating matmuls per edge
        (block-ones [128,24] stationary x [128,512] mask chunks -> PSUM
        [24,512], one partition per channel-image); ACT drains each PSUM
        with Relu+accum into counts_pe[24, edge].
      * ACT_EDGES (18): fused Sign(bias)+accum on the scalar engine
        (~5.6us/group), as in v1.
    Host differentiates the CDF and does the tiny final reduction.
  - Layout: 24 channel-images per core (4 batches x 3 ch x 2 tensors),
    packed 4 per SBUF tile as [128, 8192] bf16 -> 6 group tiles, all
    resident; input f32 streams through small staging chunks.
"""

import numpy as np

BINS = 64
N_CORES = 8
B, C, H, W = 32, 3, 512, 512
NPIX = H * W                  # 262144 per channel-image
B_LOC = B // N_CORES          # 4
IMGS = 2 * B_LOC * C          # 24 channel-images per core
PACK = 4                      # channel-images per SBUF group tile
GROUPS = IMGS // PACK         # 6
PART_PER_IMG = 128 // PACK    # 32 partitions per image
FD = NPIX // PART_PER_IMG     # 8192 free-dim elements per partition

SCALE = float(np.float32(63.0 / 128.0))              # exact in f32
BIAS2 = float(np.float32(191.0 / 128.0) - np.float32(2.0 ** -8))

# edges j=1..63; ACT (Sign+accum) takes the last N_ACT, PE the rest.
N_ACT = 18
PE_EDGES = list(range(1, BINS - N_ACT))       # 1..45
ACT_EDGES = list(range(BINS - N_ACT, BINS))   # 46..63

MM_CHUNK = 512
NCHUNK = FD // MM_CHUNK       # 16
LOAD_CHUNK = 2048

_cache = {}


def _build():
    from contextlib import ExitStack
    from concourse import bacc
    import concourse.mybir as mybir
    from concourse.tile import TileContext

    f32 = mybir.dt.float32
    bf16 = mybir.dt.bfloat16

    nc = bacc.Bacc("TRN2", target_bir_lowering=False, debug=False,
                   num_devices=N_CORES)
    x = nc.declare_dram_parameter("x", [GROUPS, 128, FD], f32, isOutput=False)
    bias_a = nc.declare_dram_parameter(
        "bias_a", [128, N_ACT], f32, isOutput=False)
    # counts_pe[i, j] = #{w >= 1 + j/64} for image i, PE-owned edge j
    counts_pe = nc.declare_dram_parameter(
        "counts_pe", [24, BINS], f32, isOutput=True)
    # counts_a[g, p, j] = sum over row of sign(w - (1+j/64-2^-9)), ACT-owned j
    counts_a = nc.declare_dram_parameter(
        "counts_a", [GROUPS, 128, BINS], f32, isOutput=True)

    with ExitStack() as es:
        tc = es.enter_context(TileContext(nc))
        pool = es.enter_context(tc.tile_pool(name="p", bufs=2))
        pp = es.enter_context(tc.tile_pool(name="pp", bufs=2, space="PSUM"))

        # persistent per-group tiles
        w_tiles = [pool.tile([128, FD], bf16, tag=f"w{g}", name=f"w{g}", bufs=1)
                   for g in range(GROUPS)]
        ones_t = [pool.tile([128, 24], bf16, tag=f"ones{g}", name=f"ones{g}", bufs=1)
                  for g in range(GROUPS)]
        for g in range(GROUPS):
            nc.gpsimd.memset(ones_t[g][:], 0.0)
            for j in range(PACK):
                nc.gpsimd.memset(
                    ones_t[g][32 * j:32 * (j + 1), 4 * g + j:4 * g + j + 1],
                    1.0)
        bt = pool.tile([128, N_ACT], f32, tag="bt", bufs=1)
        nc.sync.dma_start(out=bt[:], in_=bias_a[:])
        cnt_pe = pool.tile([24, BINS], f32, tag="cntpe", bufs=1)
        nc.vector.memset(cnt_pe[:], 0.0)
        cnt_a = [pool.tile([128, BINS], f32, tag=f"cnta{g}", name=f"cnta{g}", bufs=1)
                 for g in range(GROUPS)]
        for g in range(GROUPS):
            nc.gpsimd.memset(cnt_a[g][:], 0.0)
        act_scratch = pool.tile([128, FD], bf16, tag="acts", bufs=1)
        dump = pool.tile([24, MM_CHUNK], f32, tag="dump", bufs=1)

        # stream input in, convert to bf16 w tiles
        for g in range(GROUPS):
            for cc in range(FD // LOAD_CHUNK):
                vt = pool.tile([128, LOAD_CHUNK], f32, tag="vt", bufs=3)
                sl = slice(cc * LOAD_CHUNK, (cc + 1) * LOAD_CHUNK)
                nc.sync.dma_start(out=vt[:], in_=x[g][:, sl])
                nc.vector.tensor_scalar(
                    out=w_tiles[g][:, sl], in0=vt[:], scalar1=SCALE,
                    scalar2=BIAS2, op0=mybir.AluOpType.mult,
                    op1=mybir.AluOpType.add)

        # ACT fused jobs: (edge, group), interleaved with PE edges
        act_jobs = [(j, g) for j in ACT_EDGES for g in range(GROUPS)]
        act_emitted = 0

        def emit_act(n):
            nonlocal act_emitted
            for _ in range(n):
                if act_emitted >= len(act_jobs):
                    return
                j, g = act_jobs[act_emitted]
                k = j - ACT_EDGES[0]
                nc.scalar.activation(
                    out=act_scratch[:], in_=w_tiles[g][:],
                    func=mybir.ActivationFunctionType.Sign,
                    bias=bt[:, k:k + 1], scale=1.0,
                    accum_out=cnt_a[g][:, j:j + 1])
                act_emitted += 1

        n_pe = len(PE_EDGES)
        total_mm = GROUPS * NCHUNK
        for i, j in enumerate(PE_EDGES):
            t = float(1.0 + j / 64.0)
            ps = pp.tile([24, MM_CHUNK], f32, tag="ps")
            mm = 0
            for g in range(GROUPS):
                mk = pool.tile([128, FD], bf16, tag="mask", bufs=3)
                nc.vector.tensor_scalar(
                    out=mk[:], in0=w_tiles[g][:], scalar1=t, scalar2=None,
                    op0=mybir.AluOpType.is_ge, op1=mybir.AluOpType.bypass)
                for c in range(NCHUNK):
                    nc.tensor.matmul(
                        out=ps[:, :], lhsT=ones_t[g][:, :],
                        rhs=mk[:, c * MM_CHUNK:(c + 1) * MM_CHUNK],
                        start=(mm == 0), stop=(mm == total_mm - 1))
                    mm += 1
                target = ((i * GROUPS + g + 1) * len(act_jobs)) // (n_pe * GROUPS)
                emit_act(target - act_emitted)
            nc.scalar.activation(
                out=dump[:], in_=ps[:, :],
                func=mybir.ActivationFunctionType.Relu,
                bias=0.0, scale=1.0,
                accum_out=cnt_pe[:, j:j + 1])
        emit_act(len(act_jobs))

        nc.sync.dma_start(out=counts_pe[:], in_=cnt_pe[:])
        for g in range(GROUPS):
            nc.sync.dma_start(out=counts_a[g], in_=cnt_a[g][:])
    nc.finalize()
    return nc


def _get_nc():
    if "nc" not in _cache:
        _cache["nc"] = _build()
    return _cache["nc"]


def _pack_core(inp_c: np.ndarray, tgt_c: np.ndarray) -> np.ndarray:
    """[4,3,512,512] x2 f32 -> [GROUPS, 128, FD]; image i = t*12 + b*3 + c."""
    imgs = np.concatenate(
        [inp_c.reshape(B_LOC * C, NPIX), tgt_c.reshape(B_LOC * C, NPIX)], axis=0)
    return np.ascontiguousarray(
        imgs.reshape(GROUPS, PACK, PART_PER_IMG, FD).reshape(GROUPS, 128, FD))


def _counts_to_loss(results) -> np.float32:
    """results: list of 8 dicts with counts_pe [24, BINS] and
    counts_a [GROUPS, 128, BINS]."""
    total = np.float64(0.0)
    for c in range(N_CORES):
        cpe = np.asarray(results[c]["counts_pe"], np.float64)  # [24, BINS]
        ca = np.asarray(results[c]["counts_a"], np.float64)
        ca = ca.reshape(GROUPS, PACK, PART_PER_IMG, BINS).sum(axis=2)
        flat_a = ca.reshape(IMGS, BINS)
        cdf = np.zeros((IMGS, BINS), np.float64)
        cdf[:, 0] = NPIX
        for j in PE_EDGES:
            cdf[:, j] = cpe[:, j]
        for j in ACT_EDGES:
            cdf[:, j] = (NPIX + flat_a[:, j]) / 2.0   # sign-sum -> count_ge
        counts = np.empty((IMGS, BINS), np.float64)
        counts[:, :-1] = cdf[:, :-1] - cdf[:, 1:]
        counts[:, -1] = cdf[:, -1]
        hist = counts / NPIX   # [24, 64]; images 0..11 = input, 12..23 = target
        h_in = hist[: B_LOC * C].reshape(B_LOC, C * BINS)
        h_tg = hist[B_LOC * C:].reshape(B_LOC, C * BINS)
        total += np.abs(h_in - h_tg).sum()
    return np.float32(total / (B * C * BINS))


def _bias_np() -> np.ndarray:
    cols = [-(float(np.float32(1.0 + j / 64.0)) - 2.0 ** -9)
            for j in ACT_EDGES]
    return np.tile(np.array(cols, np.float32), (128, 1))


def _make_in_maps(input: np.ndarray, target: np.ndarray):
    inp = np.asarray(input, np.float32)
    tgt = np.asarray(target, np.float32)
    bias = _bias_np()
    in_maps = []
    for c in range(N_CORES):
        sl = slice(c * B_LOC, (c + 1) * B_LOC)
        in_maps.append({"x": _pack_core(inp[sl], tgt[sl]), "bias_a": bias})
    return in_maps


def kernel(input: np.ndarray, target: np.ndarray) -> np.ndarray:
    from concourse.bass_utils import run_bass_kernel_spmd

    nc = _get_nc()
    res = run_bass_kernel_spmd(
        nc, _make_in_maps(input, target), core_ids=list(range(N_CORES)))
    return np.asarray(_counts_to_loss(res.results), np.float32)
